# revision 2
# baseline (speedup 1.0000x reference)
"""BatchAllTripletLoss kernel for 8 Trainium2 NeuronCores.

Reference computation:
    pd = pairwise_euclidean(rep)                        # [512, 512]
    tl[a,p,k] = relu(pd[a,p] - pd[a,k] + 5.0) * mask    # [512, 512, 512]
    loss = sum(tl) / (count(tl > eps) + eps)

The mask (p!=a, k!=a, p!=k, label[p]==label[a], label[k]!=label[a])
collapses: label[p]==label[a] and label[k]!=label[a] imply p!=k and k!=a,
so valid triplets are exactly (anchor-positive pairs) x (k with a
different label).  With 64 labels over 512 rows there are only ~4100
(a,p) pairs, so instead of a dense [N,N,N] sweep each core processes its
anchors' pairs as rows of [128-pair, 512-k] tiles:

  per core (64 anchors):
    d[64,512]   = sqrt(relu(aug-matmul))            PE + DVE + ACT
    ym          = d + BIGM*same_label               DVE
    per pair-tile t:
      Gym       = sel_t.T @ ym                      PE one-hot row gather
      x[p]      = sum_k (iota==pidx)*Gym            DVE; = d[a,p] + BIGM
      xp        = x + (margin - BIGM)               DVE
      S_t[p]    = sum_k relu(xp - Gym)              ACT accum
      C_t[p]    = sum_k (Gym < xp)                  DVE accum
    out[1,2*Tp] = ones.T @ [S | C]                  PE partition sum

All matmuls run in float32r (single-pass fp32, ~2^-13 relative rounding;
the one-hot gather then carries that rounding into d).  BIGM = 128 both
masks out same-label k columns (xp <= ~35 << 128 so relu/count give
exactly 0) and carries the bias through the gather; the combined
rounding is ~1e-2 absolute per term, mean-zero, ~1e-4 on the final sums.
rep arrives both row-major (for the row-norm accumulates) and
host-transposed (pure layout permutation) so no PE transposes are
needed.  Anchors are block-sharded 64 per core; the 8 partial
(sum, count) pairs are reduced on the host (the all-reduce of the
sharding hint).  Host-side prep is integer/mask/layout logic only; all
float arithmetic runs on device.
"""

import ml_dtypes
import numpy as np

import concourse.bass as bass
import concourse.bass_utils as _bass_utils
import concourse.tile as tile
from concourse import bacc, mybir
from concourse.bass_utils import run_bass_kernel_spmd
from concourse.vector_clock import ScopedClock

# The NEFF epilogue zeroes every semaphore 0..max-sem-num across all five
# engines (~50 serialized sem-writes per engine, ~7us).  The program only
# uses sems below 176, so capping the allocator shrinks that sweep.
_orig_walrus_args = _bass_utils.get_walrus_args


def _walrus_args_small_sems(*a, **k):
    return _orig_walrus_args(*a, **k) + ["--max-sem-num=176"]


_bass_utils.get_walrus_args = _walrus_args_small_sems


_orig_aeb = bass.Bass.all_engine_barrier


def _skip_const_barrier(self, *, sem_only=False):
    if not getattr(self, "_aeb_skipped_once", False):
        self._aeb_skipped_once = True
        return
    return _orig_aeb(self, sem_only=sem_only)


def _cheap_drain_and_barrier(self, tick_clock, wait_clock):
    """Exit protocol with sequencer-only barriers: the SP drain already
    waits out every engine/DMA tick of the tile clock, so the per-engine
    pipeline drains of the stock double butterfly are redundant here."""
    drain_inst = self.nc.sync.drain()
    wait_clock.add_sem_waits(
        drain_inst.ins, ScopedClock({None: tick_clock.global_clock})
    )
    self.nc.all_engine_barrier(sem_only=True)
    popped = self.nc._tile_sem_poison_stack.pop()
    assert popped is self._sem_poison
    self.nc.clear_and_free_semaphores(list(self.sems.allocated().values()))
    self.nc.all_engine_barrier(sem_only=True)

F32 = mybir.dt.float32
F32R = mybir.dt.float32r
AF = mybir.ActivationFunctionType
OP = mybir.AluOpType

N = 512          # rows
D = 256          # embedding dim
NCORES = 8
A = N // NCORES  # anchors per core
MARGIN = 5.0
EPS = 1e-16
BIG = 1e30       # pad-pair kill value
BIGM = 128.0     # same-label mask / bias carrier (power of two)

_cache = {}


def _build(Tp: int):
    """Build the (uniform, SPMD) per-core Bass program for Tp pair tiles."""
    tile.TileContext._drain_and_barrier = _cheap_drain_and_barrier
    bass.Bass.all_engine_barrier = _skip_const_barrier
    nc = bacc.Bacc(None, target_bir_lowering=False, num_swdge_queues=2)

    rept_d = nc.declare_dram_parameter("rept", [128, 2, N], F32, isOutput=False)
    repa_d = nc.declare_dram_parameter("repa", [A, D], F32, isOutput=False)
    repat_d = nc.declare_dram_parameter("repat", [128, 2, A], F32, isOutput=False)
    bigm_d = nc.declare_dram_parameter("bigm", [A, N], mybir.dt.float8e4, isOutput=False)
    sel_d = nc.declare_dram_parameter("sel", [A, Tp * 128], mybir.dt.float8e4, isOutput=False)
    pm_d = nc.declare_dram_parameter("pm", [128, 2 * Tp], F32, isOutput=False)
    out_d = nc.declare_dram_parameter("out", [1, 2 * Tp], F32, isOutput=True)

    with tile.TileContext(nc) as tc:
        with (
            tc.tile_pool(name="singles", bufs=1) as sg,
            tc.tile_pool(name="scr", bufs=2) as scr,
            tc.tile_pool(name="xs", bufs=3) as xs,
            tc.tile_pool(name="ppf", bufs=1, space="PSUM") as ppf,
            tc.tile_pool(name="ppg", bufs=4, space="PSUM") as ppg,
            tc.tile_pool(name="ppd", bufs=1, space="PSUM") as ppd,
        ):
            iota_f = sg.tile([128, N], F32)
            nc.gpsimd.iota(
                iota_f[:], [[1, N]], channel_multiplier=0,
                allow_small_or_imprecise_dtypes=True,
            )
            ones = sg.tile([128, 1], F32)
            nc.vector.memset(ones[:], 1.0)
            onesr = sg.tile([128, 1], F32R)
            nc.vector.tensor_copy(onesr[:], ones[:])
            ones1 = sg.tile([1, A], F32)
            nc.vector.memset(ones1[:], 1.0)
            ones1r = sg.tile([1, A], F32R)
            nc.vector.tensor_copy(ones1r[:], ones1[:])
            dmy = sg.tile([1, 1], F32)
            nc.scalar.activation(dmy[:], ones[0:1, :], AF.Sqrt, bias=ones[0:1, :])

            # input loads, spread across the two HWDGE queues; rep first
            # (the row-norm chain below is the longest dependency chain)
            rept_s = sg.tile([128, 2, N], F32)     # rept[p, c, j] = rep[j, c*128+p]
            for q in range(4):
                eng = nc.sync if q % 2 == 0 else nc.scalar
                eng.dma_start(
                    rept_s[:, q // 2, (q % 2) * 256:(q % 2) * 256 + 256],
                    rept_d[:, q // 2, (q % 2) * 256:(q % 2) * 256 + 256],
                )
            repat_s = sg.tile([128, 2, A], F32)    # repat[p, c, a] = repa[a, c*128+p]
            nc.gpsimd.dma_start(repat_s[:], repat_d[:])
            repa_s = sg.tile([A, D], F32)
            nc.gpsimd.dma_start(repa_s[:], repa_d[:])
            bigm_s = sg.tile([A, N], mybir.dt.float8e4)
            nc.gpsimd.dma_start(bigm_s[:], bigm_d[:])
            sel_s = sg.tile([A, Tp * 128], mybir.dt.float8e4)
            nc.gpsimd.dma_start(sel_s[:], sel_d[:])
            pm_s = sg.tile([128, 2 * Tp], F32)     # [:, :Tp] pidx, [:, Tp:] margin
            nc.gpsimd.dma_start(pm_s[:], pm_d[:])

            # float32r operand copies (PE consumes pre-rounded data), per
            # chunk so each overlaps the other chunk's DMA
            reptr = sg.tile([128, 2, N], F32R)
            for c in range(2):
                nc.vector.tensor_copy(reptr[:, c, :], rept_s[:, c, :])
            negTa = sg.tile([128, 2, A], F32R)
            nc.vector.tensor_scalar_mul(negTa[:], repat_s[:], -2.0)

            # d2[a, j] = sq_a + sq_j - 2*dot: start the big -2*dot matmuls as
            # soon as the casts land; the sq_j rank-1 terms join the group last
            d2_p = ppd.tile([A, N], F32, tag="d2")
            nc.tensor.matmul(d2_p[:], negTa[:, 0, :], reptr[:, 0, :],
                             start=True, stop=False, skip_group_check=True)
            nc.tensor.matmul(d2_p[:], negTa[:, 1, :], reptr[:, 1, :],
                             start=False, stop=False, skip_group_check=True)

            # sq_row[1, j] = ||rep_j||^2 = ones.T @ (rept * rept)
            sqsq = sg.tile([128, 2, N], F32R)
            for c in range(2):
                nc.vector.tensor_mul(sqsq[:, c, :], rept_s[:, c, :], rept_s[:, c, :])
            sqrow_p = ppf.tile([1, N], F32, tag="fin")
            nc.tensor.matmul(sqrow_p[:], onesr[:], sqsq[:, 0, :], start=True,
                             stop=False, skip_group_check=True)
            nc.tensor.matmul(sqrow_p[:], onesr[:], sqsq[:, 1, :], start=False,
                             stop=True, skip_group_check=True)
            sqrowr = sg.tile([1, N], F32R)
            nc.vector.tensor_copy(sqrowr[:], sqrow_p[:])
            nc.tensor.matmul(d2_p[:], ones1r[:], sqrowr[:], start=False, stop=True,
                             skip_group_check=True)

            # sq_anch[64,1] = ||rep_a||^2
            sqa_scr = scr.tile([A, D], F32, tag="sqa")
            sqanch = sg.tile([A, 1], F32)
            nc.vector.scalar_tensor_tensor(
                out=sqa_scr[:], in0=repa_s[:], scalar=1.0, in1=repa_s[:],
                op0=OP.mult, op1=OP.mult, accum_out=sqanch[:],
            )

            selr = sg.tile([A, Tp * 128], F32R)
            nc.vector.tensor_copy(selr[:], sel_s[:])

            # ym = sqrt(d2 + 0.25) + BIGM*same: the +0.25 keeps the (masked)
            # diagonal's rounding noise out of sqrt's domain; its effect on
            # d_ap - d_ak cancels to ~5e-4
            sqanchb = xs.tile([A, 1], F32, tag="sqb")
            nc.vector.tensor_scalar(sqanchb[:], sqanch[:], 0.25, None, OP.add)
            dtmp = scr.tile([A, N], F32, tag="dtmp")
            nc.scalar.activation(dtmp[:], d2_p[:], AF.Sqrt, bias=sqanchb[:])
            ym = sg.tile([A, N], F32R)
            nc.vector.tensor_add(ym[:], bigm_s[:], dtmp[:])

            # pair tiles
            SC = sg.tile([128, 2 * Tp], F32)
            nc.vector.memset(SC[:], 0.0)
            relbig = sg.tile([128, Tp, N], F32)
            for t in range(Tp):
                gy = ppg.tile([128, N], F32, tag="gy")
                nc.tensor.matmul(gy[:], selr[:, t * 128:(t + 1) * 128], ym[:],
                                 start=True, stop=True)

                stt = scr.tile([128, N], F32, tag="stt")
                xv = xs.tile([128, 1], F32, tag="xv")
                nc.vector.scalar_tensor_tensor(
                    out=stt[:], in0=iota_f[:], scalar=pm_s[:, t:t + 1], in1=gy[:],
                    op0=OP.is_equal, op1=OP.mult, accum_out=xv[:],
                )
                xp = xs.tile([128, 1], F32, tag="xp")
                nc.vector.tensor_scalar(
                    xp[:], xv[:], pm_s[:, Tp + t:Tp + t + 1], None, OP.add
                )

                nc.scalar.activation(
                    relbig[:, t, :], gy[:], AF.Relu, bias=xp[:], scale=-1.0,
                    accum_out=SC[:, t:t + 1],
                )

            # counts: relu output is positive exactly where a triplet is
            # positive, so two wide scans replace five per-tile ones
            h = (Tp + 1) // 2
            nc.vector.tensor_scalar(
                relbig[:, 0:h, :], relbig[:, 0:h, :], 0.0, 0.0, OP.is_gt, OP.add,
                accum_out=SC[:, Tp:Tp + 1],
            )
            if Tp > h:
                nc.vector.tensor_scalar(
                    relbig[:, h:Tp, :], relbig[:, h:Tp, :], 0.0, 0.0,
                    OP.is_gt, OP.add,
                    accum_out=SC[:, Tp + 1:Tp + 2],
                )

            # partition-sum S and C columns -> [1, 2*Tp]
            fin_p = ppf.tile([1, 2 * Tp], F32, tag="fin")
            nc.tensor.matmul(fin_p[:], ones[:], SC[:], start=True, stop=True)
            outsb = sg.tile([1, 2 * Tp], F32)
            nc.vector.tensor_copy(outsb[:], fin_p[:])
            nc.sync.dma_start(out_d[:], outsb[:])

    nc.finalize()
    return nc


def _prep(rep: np.ndarray, labels: np.ndarray):
    """Host-side integer/mask/layout prep: shard anchors, enumerate pairs."""
    rep = np.ascontiguousarray(np.asarray(rep, dtype=np.float32))
    labels = np.asarray(labels)
    same = labels[:, None] == labels[None, :]

    # rep.T packed [128, 2, N]: rept[p, c, j] = rep[j, c*128 + p]
    rept = np.ascontiguousarray(
        rep.T.reshape(2, 128, N).transpose(1, 0, 2)
    )

    pairs = []
    for c in range(NCORES):
        base = c * A
        prs = [
            (j, p)
            for j in range(A)
            for p in np.nonzero(same[base + j])[0]
            if p != base + j
        ]
        pairs.append(prs)
    Tp = max(1, max((len(p) + 127) // 128 for p in pairs))

    in_maps = []
    for c in range(NCORES):
        base = c * A
        repa = rep[base:base + A]
        repat = np.ascontiguousarray(
            repa.T.reshape(2, 128, A).transpose(1, 0, 2)
        )
        bigm = np.where(same[base:base + A], BIGM, 0.0).astype(ml_dtypes.float8_e4m3)
        sel = np.zeros((A, Tp * 128), ml_dtypes.float8_e4m3)
        pm = np.zeros((128, 2 * Tp), np.float32)
        pm[:, Tp:] = -BIG
        for i, (j, p) in enumerate(pairs[c]):
            t, r = divmod(i, 128)
            sel[j, i] = 1.0
            pm[r, t] = p
            pm[r, Tp + t] = MARGIN - BIGM
        in_maps.append({
            "rept": rept,
            "repa": repa,
            "repat": repat,
            "bigm": bigm,
            "sel": sel,
            "pm": pm,
        })
    return Tp, in_maps


def _run(rep, labels, trace=False):
    Tp, in_maps = _prep(rep, labels)
    if Tp not in _cache:
        _cache[Tp] = _build(Tp)
    nc = _cache[Tp]
    res = run_bass_kernel_spmd(nc, in_maps, list(range(NCORES)), trace=trace)
    outs = np.stack([res.results[c]["out"][0] for c in range(NCORES)])  # [8, 2*Tp]
    S = float(outs[:, :Tp].sum())
    C = float(outs[:, Tp:].sum())
    loss = np.float32(S / (C + EPS))
    return np.asarray(loss, dtype=np.float32), res


def kernel(rep, labels):
    loss, _ = _run(rep, labels, trace=False)
    return loss



# revision 11
# speedup vs baseline: 1.0372x; 1.0372x over previous
"""BatchAllTripletLoss kernel for 8 Trainium2 NeuronCores.

Reference computation:
    pd = pairwise_euclidean(rep)                        # [512, 512]
    tl[a,p,k] = relu(pd[a,p] - pd[a,k] + 5.0) * mask    # [512, 512, 512]
    loss = sum(tl) / (count(tl > eps) + eps)

The mask (p!=a, k!=a, p!=k, label[p]==label[a], label[k]!=label[a])
collapses: valid triplets are (anchor-positive pairs) x (k with a
different label).  With 64 labels over 512 rows there are ~4100 (a,p)
pairs (~512 per core of 64 anchors), so instead of a dense [N,N,N]
sweep each core scans its pairs as rows of [128, 512-k] tiles.

The whole distance pipeline runs in bf16 (margin 5 dominates; the bf16
noise of ~0.25 on d+BIGM perturbs the loss by ~1e-3, far inside the
2e-2 gate):

  d[64,512]   = sqrt(-2*(dot - sq_k/2) + sq_a + .25)   PE group + ACT
  ym          = d + BIGM*same_label                    DVE (bf16)
  ymdup[128,·]= dupsel.T @ ym  (row r -> its anchor)   PE + DVE copy
  xpd[128,Tp] = ymdup[r, pidx[r,t]]                    Pool indirect gather
  xpm         = xpd + (margin - BIGM | -BIG pad)       DVE
  tile t (DVE): scr = min(ym - xp, 0), accum -> -S_t   DVE bf16 4x
  tile t (ACT): scr = relu(xp - ym),  accum -> +S_t    ACT from PSUM
  counts      : (scr<0)/(scr>0) wide scans, accum      DVE bf16 4x
  out[2,·]    = [-1|+1] ones.T @ SC                    PE partition sum

Rows are packed per-anchor (each row holds up to Tp same-anchor pairs)
so the pair-distance extraction is a per-partition gather instead of a
per-tile 512-wide is_equal scan.  BIGM = 128 both masks same-label k
columns and carries the bias through the gather.  Anchors are
block-sharded 64 per core; the 8 partial (S, C) pairs are reduced on
the host (the all-reduce of the sharding hint).  Host-side prep is
integer/mask/layout work plus dtype casts; all float arithmetic runs
on device.
"""

import ml_dtypes
import numpy as np

import concourse.bass as bass
import concourse.tile as tile
from concourse import bacc, mybir
from concourse.bass_utils import run_bass_kernel_spmd
from concourse.vector_clock import ScopedClock


_orig_aeb = bass.Bass.all_engine_barrier


def _skip_const_barrier(self, *, sem_only=False):
    if not getattr(self, "_aeb_skipped_once", False):
        self._aeb_skipped_once = True
        return
    return _orig_aeb(self, sem_only=sem_only)


def _cheap_drain_and_barrier(self, tick_clock, wait_clock):
    """Exit protocol with sequencer-only barriers: the SP drain already
    waits out every engine/DMA tick of the tile clock, so the per-engine
    pipeline drains of the stock double butterfly are redundant here."""
    drain_inst = self.nc.sync.drain()
    wait_clock.add_sem_waits(
        drain_inst.ins, ScopedClock({None: tick_clock.global_clock})
    )
    self.nc.all_engine_barrier(sem_only=True)
    popped = self.nc._tile_sem_poison_stack.pop()
    assert popped is self._sem_poison
    self.nc.clear_and_free_semaphores(list(self.sems.allocated().values()))
    self.nc.all_engine_barrier(sem_only=True)


F32 = mybir.dt.float32
BF16 = mybir.dt.bfloat16
U16 = mybir.dt.uint16
AF = mybir.ActivationFunctionType
OP = mybir.AluOpType

N = 512          # rows
D = 256          # embedding dim
NCORES = 8
A = N // NCORES  # anchors per core
MARGIN = 5.0
EPS = 1e-16
BIG = 1e30       # pad-slot kill value
BIGM = 128.0     # same-label mask / bias carrier (power of two)

_cache = {}


def _build(Tp: int, nact: int):
    """Build the (uniform, SPMD) per-core Bass program.

    Tp slots per row; the last `nact` slots run their S-scan on the ACT
    engine (relu from PSUM), the first Tp-nact on DVE (min-scan, sign
    flipped by the final matmul).
    """
    ndve = Tp - nact
    tile.TileContext._drain_and_barrier = _cheap_drain_and_barrier
    bass.Bass.all_engine_barrier = _skip_const_barrier
    nc = bacc.Bacc(None, target_bir_lowering=False, num_swdge_queues=2)

    rept_d = nc.declare_dram_parameter("rept", [128, 2, N], BF16, isOutput=False)
    repat_d = nc.declare_dram_parameter("repat", [128, 2, A], BF16, isOutput=False)
    repa_d = nc.declare_dram_parameter("repa", [A, D], BF16, isOutput=False)
    bigm_d = nc.declare_dram_parameter("bigm", [A, N], BF16, isOutput=False)
    dups_d = nc.declare_dram_parameter("dups", [A, 128], BF16, isOutput=False)
    idx_d = nc.declare_dram_parameter("idx", [128, Tp], U16, isOutput=False)
    msk_d = nc.declare_dram_parameter("msk", [128, Tp, 16], BF16, isOutput=False)
    pmadj_d = nc.declare_dram_parameter("pmadj", [128, Tp], F32, isOutput=False)
    out_d = nc.declare_dram_parameter("out", [2, Tp + 2], F32, isOutput=True)

    with tile.TileContext(nc) as tc:
        with (
            tc.tile_pool(name="singles", bufs=1) as sg,
            tc.tile_pool(name="scr", bufs=1) as scrp,
            tc.tile_pool(name="ppd", bufs=1, space="PSUM") as ppd,
            tc.tile_pool(name="ppy", bufs=1, space="PSUM") as ppy,
            tc.tile_pool(name="ppf", bufs=1, space="PSUM") as ppf,
        ):
            ones = sg.tile([128, 1], F32)
            nc.vector.memset(ones[:], 1.0)
            zerosb = sg.tile([128, N], BF16)
            nc.vector.memset(zerosb[:], 0.0)
            dmy = sg.tile([1, 1], F32)
            nc.scalar.activation(dmy[:], ones[0:1, :], AF.Sqrt, bias=ones[0:1, :])
            nc.scalar.activation(dmy[:], ones[0:1, :], AF.Relu, bias=ones[0:1, :])

            onesb = sg.tile([128, A], BF16)
            nc.gpsimd.memset(onesb[:], 1.0)
            pmones = sg.tile([128, 2], F32)
            nc.gpsimd.memset(pmones[:, 0:1], -1.0)
            nc.gpsimd.memset(pmones[:, 1:2], 1.0)
            SC = sg.tile([128, Tp + 2], F32)
            nc.gpsimd.memset(SC[:], 0.0)

            # input loads: repat + rept c0 first (the d2 group's critical
            # operands), small index tensors behind them on each queue
            repat_s = sg.tile([128, 2, A], BF16)
            nc.sync.dma_start(repat_s[:], repat_d[:])
            rept_s = sg.tile([128, 2, N], BF16)
            nc.sync.dma_start(rept_s[:, 0, :], rept_d[:, 0, :])
            nc.scalar.dma_start(rept_s[:, 1, :], rept_d[:, 1, :])
            repa_s = sg.tile([A, D], BF16)
            nc.gpsimd.dma_start(repa_s[:], repa_d[:])
            bigm_s = sg.tile([A, N], BF16)
            nc.gpsimd.dma_start(bigm_s[:], bigm_d[:])
            dups_s = sg.tile([A, 128], BF16)
            nc.scalar.dma_start(dups_s[:], dups_d[:])
            idx_s = sg.tile([128, Tp], U16)
            nc.sync.dma_start(idx_s[:], idx_d[:])
            pmadj_s = sg.tile([128, Tp], F32)
            nc.sync.dma_start(pmadj_s[:], pmadj_d[:])
            msk_s = sg.tile([128, Tp, 16], BF16)
            nc.scalar.dma_start(msk_s[:], msk_d[:])

            # sqsq[p, c, j] = -0.5 * rept^2 ; summed into the d2 group by
            # the ones-matmuls so sqrt's scale=-2 yields sq_k - 2 dot
            sqsq = sg.tile([128, 2, N], BF16)
            for c in range(2):
                nc.vector.scalar_tensor_tensor(
                    out=sqsq[:, c, :], in0=rept_s[:, c, :], scalar=-0.5,
                    in1=rept_s[:, c, :], op0=OP.mult, op1=OP.mult,
                )

            # sq_anch[64,1] (+0.25 sqrt-domain guard, as in the reference
            # EPS trick; the bias rides the ACT activation)
            sqa_scr = scrp.tile([A, D], BF16, tag="sqa")
            sqanch = sg.tile([A, 1], F32)
            nc.vector.scalar_tensor_tensor(
                out=sqa_scr[:], in0=repa_s[:], scalar=1.0, in1=repa_s[:],
                op0=OP.mult, op1=OP.mult, accum_out=sqanch[:],
            )
            sqanchb = sg.tile([A, 1], F32)
            nc.vector.tensor_scalar(sqanchb[:], sqanch[:], 0.25, None, OP.add)

            # d2 group: acc = dot - 0.5*sq_k  (PE order: c0 dot, c0 sq,
            # c1 dot, c1 sq -- each starts as soon as its operands land)
            d2_p = ppd.tile([A, N], F32, tag="d2")
            nc.tensor.matmul(d2_p[:], repat_s[:, 0, :], rept_s[:, 0, :],
                             start=True, stop=False, skip_group_check=True)
            nc.tensor.matmul(d2_p[:], onesb[:], sqsq[:, 0, :],
                             start=False, stop=False, skip_group_check=True)
            nc.tensor.matmul(d2_p[:], repat_s[:, 1, :], rept_s[:, 1, :],
                             start=False, stop=False, skip_group_check=True)
            nc.tensor.matmul(d2_p[:], onesb[:], sqsq[:, 1, :],
                             start=False, stop=True, skip_group_check=True)

            # d = sqrt(-2*acc + sq_a + .25), then ym = d + BIGM*same
            d_sb = sg.tile([A, N], BF16)
            nc.scalar.activation(d_sb[:], d2_p[:], AF.Sqrt, bias=sqanchb[:],
                                 scale=-2.0)
            ym = sg.tile([A, N], BF16)
            nc.vector.tensor_add(ym[:], bigm_s[:], d_sb[:])

            # duplicate anchor rows out to their pair rows
            ymdup_p = ppy.tile([128, N], F32, tag="ymdup")
            nc.tensor.matmul(ymdup_p[:], dups_s[:], ym[:], start=True, stop=True)
            ymdup = sg.tile([128, N], BF16)
            nc.vector.tensor_copy(ymdup[:], ymdup_p[:])

            # per-slot pair distance (d+BIGM): the Pool gather uses one
            # column list per 16-partition group (the col-major unwrap of
            # idx, i.e. G[s*16+q] = idx[lo+q, s]), so gather all 16
            # partners' columns and take the q == p%16 diagonal via a
            # host mask + X-axis reduce
            gat = sg.tile([128, Tp, 16], BF16)
            nc.gpsimd.indirect_copy(
                gat[:].rearrange("p a b -> p (a b)"), ymdup[:], idx_s[:], True
            )
            gatm = sg.tile([128, Tp, 16], BF16)
            nc.vector.tensor_mul(gatm[:], gat[:], msk_s[:])
            xpd = sg.tile([128, Tp], F32)
            nc.vector.tensor_reduce(xpd[:], gatm[:], mybir.AxisListType.X, OP.add)
            xpm = sg.tile([128, Tp], F32)
            nc.vector.tensor_add(xpm[:], xpd[:], pmadj_s[:])

            scratch = sg.tile([128, Tp, N], BF16)
            # ACT slots: relu(xp - ym) from PSUM, accum -> +S_t
            for t in range(ndve, Tp):
                nc.scalar.activation(
                    scratch[:, t, :], ymdup_p[:], AF.Relu,
                    bias=xpm[:, t:t + 1], scale=-1.0,
                    accum_out=SC[:, t:t + 1],
                )
            # DVE slots: min(ym - xp, 0), accum -> -S_t (bf16 4x mode;
            # STT's accum is an add-reduce of the post-op1 output)
            for t in range(ndve):
                nc.vector.scalar_tensor_tensor(
                    out=scratch[:, t, :], in0=ymdup[:], scalar=xpm[:, t:t + 1],
                    in1=zerosb[:], op0=OP.subtract, op1=OP.min,
                    accum_out=SC[:, t:t + 1],
                )

            # counts: one wide scan per engine-group (predicates differ:
            # DVE scratch is <=0 with positives strictly negative; ACT
            # scratch is >=0 with positives strictly positive)
            if ndve > 0:
                nc.vector.tensor_scalar(
                    scratch[:, 0:ndve, :], scratch[:, 0:ndve, :], 0.0, 0.0,
                    OP.is_lt, OP.add, accum_out=SC[:, Tp:Tp + 1],
                )
            if nact > 0:
                nc.vector.tensor_scalar(
                    scratch[:, ndve:Tp, :], scratch[:, ndve:Tp, :], 0.0, 0.0,
                    OP.is_gt, OP.add, accum_out=SC[:, Tp + 1:Tp + 2],
                )

            # partition-sum with both signs: row 0 = -sum, row 1 = +sum
            fin_p = ppf.tile([2, Tp + 2], F32, tag="fin")
            nc.tensor.matmul(fin_p[:], pmones[:], SC[:], start=True, stop=True)
            outsb = sg.tile([2, Tp + 2], F32)
            nc.vector.tensor_copy(outsb[:], fin_p[:])
            nc.sync.dma_start(out_d[:], outsb[:])

    nc.finalize()
    return nc


def _prep(rep: np.ndarray, labels: np.ndarray):
    """Host-side prep: shard anchors, bin-pack pairs into per-anchor rows."""
    rep = np.asarray(rep, dtype=np.float32)
    labels = np.asarray(labels)
    same = labels[:, None] == labels[None, :]
    repb = rep.astype(ml_dtypes.bfloat16)

    # rep.T packed [128, 2, N]: rept[p, c, j] = rep[j, c*128 + p]
    rept = np.ascontiguousarray(
        repb.T.reshape(2, 128, N).transpose(1, 0, 2)
    )

    core_pairs = []      # per core: list over anchors of pair-index lists
    for c in range(NCORES):
        base = c * A
        plists = []
        for j in range(A):
            ps = [int(p) for p in np.nonzero(same[base + j])[0] if p != base + j]
            plists.append(ps)
        core_pairs.append(plists)

    def rows_needed(plists, T):
        return sum((len(ps) + T - 1) // T for ps in plists)

    Tp = 1
    while any(rows_needed(pl, Tp) > 128 for pl in core_pairs):
        Tp += 1
    nact = min(3, max(1, Tp - 3))

    in_maps = []
    for c in range(NCORES):
        base = c * A
        repa = repb[base:base + A]
        repat = np.ascontiguousarray(
            repa.T.reshape(2, 128, A).transpose(1, 0, 2)
        )
        bigm = np.where(same[base:base + A], BIGM, 0.0).astype(ml_dtypes.bfloat16)
        dups = np.zeros((A, 128), ml_dtypes.bfloat16)
        idx = np.zeros((128, Tp), np.uint16)
        msk = np.zeros((128, Tp, 16), ml_dtypes.bfloat16)
        for p in range(128):
            msk[p, :, p % 16] = 1.0
        pmadj = np.full((128, Tp), -BIG, np.float32)
        r = 0
        for j, ps in enumerate(core_pairs[c]):
            for s in range(0, len(ps), Tp):
                chunk = ps[s:s + Tp]
                dups[j, r] = 1.0
                for t, p in enumerate(chunk):
                    idx[r, t] = p
                    pmadj[r, t] = MARGIN - BIGM
                r += 1
        assert r <= 128, (c, r)
        in_maps.append({
            "rept": rept,
            "repat": repat,
            "repa": repa,
            "bigm": bigm,
            "dups": dups,
            "idx": idx,
            "msk": msk,
            "pmadj": pmadj,
        })
    return Tp, nact, in_maps


def _run(rep, labels, trace=False):
    Tp, nact, in_maps = _prep(rep, labels)
    ndve = Tp - nact
    if (Tp, nact) not in _cache:
        _cache[(Tp, nact)] = _build(Tp, nact)
    nc = _cache[(Tp, nact)]
    res = run_bass_kernel_spmd(nc, in_maps, list(range(NCORES)), trace=trace)
    outs = np.stack([res.results[c]["out"] for c in range(NCORES)])  # [8, 2, Tp+2]
    S = float(outs[:, 0, :ndve].sum()) + float(outs[:, 1, ndve:Tp].sum())
    C = float(outs[:, 1, Tp:].sum())
    loss = np.float32(S / (C + EPS))
    return np.asarray(loss, dtype=np.float32), res


def kernel(rep, labels):
    loss, _ = _run(rep, labels, trace=False)
    return loss


# revision 19
# speedup vs baseline: 1.0767x; 1.0380x over previous
"""BatchAllTripletLoss kernel for 8 Trainium2 NeuronCores.

Reference computation:
    pd = pairwise_euclidean(rep)                        # [512, 512]
    tl[a,p,k] = relu(pd[a,p] - pd[a,k] + 5.0) * mask    # [512, 512, 512]
    loss = sum(tl) / (count(tl > eps) + eps)

The mask (p!=a, k!=a, p!=k, label[p]==label[a], label[k]!=label[a])
collapses: valid triplets are (anchor-positive pairs) x (k with a
different label).  With 64 labels over 512 rows there are ~4100 (a,p)
pairs (~512 per core of 64 anchors), so instead of a dense [N,N,N]
sweep each core scans its pairs as rows of [128, 512-k] tiles.

The whole distance pipeline runs in bf16 (margin 5 dominates; the bf16
noise of ~0.25 on d+BIGM perturbs the loss by ~1e-3, far inside the
2e-2 gate):

  d[64,512]   = sqrt(-2*(dot - sq_k/2) + sq_a + .25)   PE group + ACT
  ym          = d + BIGM*same_label                    DVE (bf16)
  ymdup[128,·]= dupsel.T @ ym  (row r -> its anchor)   PE + DVE copy
  xpd[128,Tp] = ymdup[r, pidx[r,t]]                    Pool indirect gather
  xpm         = xpd + (margin - BIGM | -BIG pad)       DVE
  tile t (DVE): scr = min(ym - xp, 0), accum -> -S_t   DVE bf16 4x
  tile t (ACT): scr = relu(xp - ym),  accum -> +S_t    ACT from PSUM
  counts      : (scr<0)/(scr>0) wide scans, accum      DVE bf16 4x
  out[2,·]    = [-1|+1] ones.T @ SC                    PE partition sum

Rows are packed per-anchor (each row holds up to Tp same-anchor pairs)
so the pair-distance extraction is a per-partition gather instead of a
per-tile 512-wide is_equal scan.  BIGM = 128 both masks same-label k
columns and carries the bias through the gather.  Anchors are
block-sharded 64 per core; the 8 partial (S, C) pairs are reduced on
the host (the all-reduce of the sharding hint).  Host-side prep is
integer/mask/layout work plus dtype casts; all float arithmetic runs
on device.
"""

import ml_dtypes
import numpy as np

import concourse.bass as bass
import concourse.tile as tile
from concourse import bacc, mybir
from concourse.bass_utils import run_bass_kernel_spmd
from concourse.vector_clock import ScopedClock


_orig_aeb = bass.Bass.all_engine_barrier


def _skip_const_barrier(self, *, sem_only=False):
    if not getattr(self, "_aeb_skipped_once", False):
        self._aeb_skipped_once = True
        return
    return _orig_aeb(self, sem_only=sem_only)


def _cheap_drain_and_barrier(self, tick_clock, wait_clock):
    """Exit protocol with sequencer-only barriers: the SP drain already
    waits out every engine/DMA tick of the tile clock, so the per-engine
    pipeline drains of the stock double butterfly are redundant here."""
    drain_inst = self.nc.sync.drain()
    wait_clock.add_sem_waits(
        drain_inst.ins, ScopedClock({None: tick_clock.global_clock})
    )
    self.nc.all_engine_barrier(sem_only=True)
    popped = self.nc._tile_sem_poison_stack.pop()
    assert popped is self._sem_poison
    self.nc.clear_and_free_semaphores(list(self.sems.allocated().values()))
    self.nc.all_engine_barrier(sem_only=True)


F32 = mybir.dt.float32
BF16 = mybir.dt.bfloat16
U16 = mybir.dt.uint16
AF = mybir.ActivationFunctionType
OP = mybir.AluOpType

N = 512          # rows
D = 256          # embedding dim
NCORES = 8
A = N // NCORES  # anchors per core
MARGIN = 5.0
EPS = 1e-16
BIG = 1e30       # pad-slot kill value
BIGM = 128.0     # same-label mask / bias carrier (power of two)

_cache = {}


def _build(Tp: int, nact: int):
    """Build the (uniform, SPMD) per-core Bass program.

    Tp slots per row; the last `nact` slots run their S-scan on the ACT
    engine (relu from PSUM), the first Tp-nact on DVE (min-scan, sign
    flipped by the final matmul).
    """
    ndve = Tp - nact
    tile.TileContext._drain_and_barrier = _cheap_drain_and_barrier
    bass.Bass.all_engine_barrier = _skip_const_barrier
    nc = bacc.Bacc(None, target_bir_lowering=False, num_swdge_queues=2)

    rept_d = nc.declare_dram_parameter("rept", [128, 2, N], BF16, isOutput=False)
    repat_d = nc.declare_dram_parameter("repat", [128, 2, A], BF16, isOutput=False)
    repa_d = nc.declare_dram_parameter("repa", [A, D], BF16, isOutput=False)
    bigm_d = nc.declare_dram_parameter("bigm", [A, N], BF16, isOutput=False)
    dups_d = nc.declare_dram_parameter("dups", [A, 128], BF16, isOutput=False)
    idx_d = nc.declare_dram_parameter("idx", [128, Tp], U16, isOutput=False)
    msk_d = nc.declare_dram_parameter("msk", [128, Tp, 16], BF16, isOutput=False)
    pmadj_d = nc.declare_dram_parameter("pmadj", [128, Tp], F32, isOutput=False)
    NW = Tp + 1 + nact
    out_d = nc.declare_dram_parameter("out", [2, NW], F32, isOutput=True)

    with tile.TileContext(nc) as tc:
        with (
            tc.tile_pool(name="singles", bufs=1) as sg,
            tc.tile_pool(name="scr", bufs=1) as scrp,
            tc.tile_pool(name="ppd", bufs=1, space="PSUM") as ppd,
            tc.tile_pool(name="ppy", bufs=1, space="PSUM") as ppy,
            tc.tile_pool(name="ppf", bufs=1, space="PSUM") as ppf,
            tc.tile_pool(name="ppw", bufs=1, space="PSUM") as ppw,
        ):
            ones = sg.tile([128, 1], F32)
            nc.vector.memset(ones[:], 1.0)
            zerosb = sg.tile([128, N], BF16)
            nc.vector.memset(zerosb[:], 0.0)
            dmy = sg.tile([1, 1], F32)
            nc.scalar.activation(dmy[:], ones[0:1, :], AF.Sqrt, bias=ones[0:1, :])
            nc.scalar.activation(dmy[:], ones[0:1, :], AF.Relu, bias=ones[0:1, :])

            onesb = sg.tile([128, A], BF16)
            nc.vector.memset(onesb[:], 1.0)
            pmones = sg.tile([128, 2], F32)
            nc.vector.memset(pmones[:, 0:1], -1.0)
            nc.vector.memset(pmones[:, 1:2], 1.0)
            SC = sg.tile([128, NW], F32)
            nc.vector.memset(SC[:], 0.0)

            # preload the gpsimd ucode library (indirect_copy) during the
            # DMA window instead of on the xp critical path
            dmyi = sg.tile([128, 4], U16)
            nc.gpsimd.memset(dmyi[:], 0)
            dmyg = sg.tile([128, 4], BF16)
            nc.gpsimd.indirect_copy(dmyg[:], zerosb[:], dmyi[:], True)

            # input loads: rept c0 + repat first (the d2 group's critical
            # operands), late-needed tensors behind them on each queue
            rept_s = sg.tile([128, 2, N], BF16)
            nc.sync.dma_start(rept_s[:, 0, :], rept_d[:, 0, :])
            repat_s = sg.tile([128, 2, A], BF16)
            nc.gpsimd.dma_start(repat_s[:], repat_d[:])
            nc.scalar.dma_start(rept_s[:, 1, :], rept_d[:, 1, :])
            repa_s = sg.tile([A, D], BF16)
            nc.gpsimd.dma_start(repa_s[:], repa_d[:])
            bigm_s = sg.tile([A, N], BF16)
            nc.gpsimd.dma_start(bigm_s[:], bigm_d[:])
            idx_s = sg.tile([128, Tp], U16)
            nc.sync.dma_start(idx_s[:], idx_d[:])
            pmadj_s = sg.tile([128, Tp], F32)
            nc.sync.dma_start(pmadj_s[:], pmadj_d[:])
            msk_s = sg.tile([128, Tp, 16], BF16)
            nc.scalar.dma_start(msk_s[:], msk_d[:])
            dups_s = sg.tile([A, 128], BF16)
            nc.scalar.dma_start(dups_s[:], dups_d[:])

            # warm the PE p-state during the DMA window so the real matmul
            # chain runs at full clock
            junk_p = ppw.tile([A, N], F32, tag="warm")
            for w in range(3):
                nc.tensor.matmul(junk_p[:], onesb[:], zerosb[:],
                                 start=(w == 0), stop=(w == 2),
                                 skip_group_check=True)

            # sqsq[p, c, j] = -0.5 * rept^2 ; summed into the d2 group by
            # the ones-matmuls so sqrt's scale=-2 yields sq_k - 2 dot
            sqsq = sg.tile([128, 2, N], BF16)
            for c in range(2):
                nc.vector.scalar_tensor_tensor(
                    out=sqsq[:, c, :], in0=rept_s[:, c, :], scalar=-0.5,
                    in1=rept_s[:, c, :], op0=OP.mult, op1=OP.mult,
                )

            # sq_anch[64,1] (+0.25 sqrt-domain guard, as in the reference
            # EPS trick; the bias rides the ACT activation)
            sqa_scr = scrp.tile([A, D], BF16, tag="sqa")
            sqanch = sg.tile([A, 1], F32)
            nc.vector.scalar_tensor_tensor(
                out=sqa_scr[:], in0=repa_s[:], scalar=1.0, in1=repa_s[:],
                op0=OP.mult, op1=OP.mult, accum_out=sqanch[:],
            )
            sqanchb = sg.tile([A, 1], F32)
            nc.vector.tensor_scalar(sqanchb[:], sqanch[:], 0.25, None, OP.add)

            # d2 group: acc = dot - 0.5*sq_k  (PE order: c0 dot, c0 sq,
            # c1 dot, c1 sq -- each starts as soon as its operands land)
            d2_p = ppd.tile([A, N], F32, tag="d2")
            nc.tensor.matmul(d2_p[:], repat_s[:, 0, :], rept_s[:, 0, :],
                             start=True, stop=False, skip_group_check=True)
            nc.tensor.matmul(d2_p[:], onesb[:], sqsq[:, 0, :],
                             start=False, stop=False, skip_group_check=True)
            nc.tensor.matmul(d2_p[:], repat_s[:, 1, :], rept_s[:, 1, :],
                             start=False, stop=False, skip_group_check=True)
            nc.tensor.matmul(d2_p[:], onesb[:], sqsq[:, 1, :],
                             start=False, stop=True, skip_group_check=True)

            # d = sqrt(-2*acc + sq_a + .25), then ym = d + BIGM*same
            d_sb = sg.tile([A, N], BF16)
            nc.scalar.activation(d_sb[:], d2_p[:], AF.Sqrt, bias=sqanchb[:],
                                 scale=-2.0)
            ym = sg.tile([A, N], BF16)
            nc.vector.tensor_add(ym[:], bigm_s[:], d_sb[:])

            # duplicate anchor rows out to their pair rows
            ymdup_p = ppy.tile([128, N], F32, tag="ymdup")
            nc.tensor.matmul(ymdup_p[:], dups_s[:], ym[:], start=True, stop=True)
            ymdup = sg.tile([128, N], BF16)
            nc.scalar.activation(ymdup[:], ymdup_p[:], AF.Copy)

            # per-slot pair distance (d+BIGM): the Pool gather uses one
            # column list per 16-partition group (the col-major unwrap of
            # idx, i.e. G[s*16+q] = idx[lo+q, s]), so gather all 16
            # partners' columns and take the q == p%16 diagonal via a
            # host mask + X-axis reduce
            gat = sg.tile([128, Tp, 16], BF16)
            nc.gpsimd.indirect_copy(
                gat[:].rearrange("p a b -> p (a b)"), ymdup[:], idx_s[:], True
            )
            gatm = sg.tile([128, Tp, 16], BF16)
            nc.vector.tensor_mul(gatm[:], gat[:], msk_s[:])
            xpd = sg.tile([128, Tp], F32)
            nc.vector.tensor_reduce(xpd[:], gatm[:], mybir.AxisListType.X, OP.add)
            xpm = sg.tile([128, Tp], F32)
            nc.vector.tensor_add(xpm[:], xpd[:], pmadj_s[:])

            scratch = sg.tile([128, Tp, N], BF16)
            # ACT slots: relu(xp - ym) from PSUM, accum -> +S_t
            for t in range(ndve, Tp):
                nc.scalar.activation(
                    scratch[:, t, :], ymdup_p[:], AF.Relu,
                    bias=xpm[:, t:t + 1], scale=-1.0,
                    accum_out=SC[:, t:t + 1],
                )
            # DVE slots: min(ym - xp, 0), accum -> -S_t (bf16 4x mode;
            # STT's accum is an add-reduce of the post-op1 output)
            for t in range(ndve):
                nc.vector.scalar_tensor_tensor(
                    out=scratch[:, t, :], in0=ymdup[:], scalar=xpm[:, t:t + 1],
                    in1=zerosb[:], op0=OP.subtract, op1=OP.min,
                    accum_out=SC[:, t:t + 1],
                )

            # counts, all on DVE: one merged scan over the DVE tiles
            # (scratch <= 0, positives strictly negative), then per-tile
            # scans of the ACT tiles as each relu lands (scratch >= 0,
            # positives strictly positive) so DVE never idle-waits ACT
            if ndve > 0:
                nc.vector.tensor_scalar(
                    scratch[:, 0:ndve, :], scratch[:, 0:ndve, :], 0.0, 0.0,
                    OP.is_lt, OP.add, accum_out=SC[:, Tp:Tp + 1],
                )
            for i, t in enumerate(range(ndve, Tp)):
                nc.vector.tensor_scalar(
                    scratch[:, t, :], scratch[:, t, :], 0.0, 0.0,
                    OP.is_gt, OP.add, accum_out=SC[:, Tp + 1 + i:Tp + 2 + i],
                )

            # partition-sum with both signs: row 0 = -sum, row 1 = +sum
            fin_p = ppf.tile([2, NW], F32, tag="fin")
            nc.tensor.matmul(fin_p[:], pmones[:], SC[:], start=True, stop=True)
            outsb = sg.tile([2, NW], F32)
            nc.vector.tensor_copy(outsb[:], fin_p[:])
            nc.sync.dma_start(out_d[:], outsb[:])

    nc.finalize()
    return nc


def _prep(rep: np.ndarray, labels: np.ndarray):
    """Host-side prep: shard anchors, bin-pack pairs into per-anchor rows."""
    rep = np.asarray(rep, dtype=np.float32)
    labels = np.asarray(labels)
    same = labels[:, None] == labels[None, :]
    repb = rep.astype(ml_dtypes.bfloat16)

    # rep.T packed [128, 2, N]: rept[p, c, j] = rep[j, c*128 + p]
    rept = np.ascontiguousarray(
        repb.T.reshape(2, 128, N).transpose(1, 0, 2)
    )

    core_pairs = []      # per core: list over anchors of pair-index lists
    for c in range(NCORES):
        base = c * A
        plists = []
        for j in range(A):
            ps = [int(p) for p in np.nonzero(same[base + j])[0] if p != base + j]
            plists.append(ps)
        core_pairs.append(plists)

    def rows_needed(plists, T):
        return sum((len(ps) + T - 1) // T for ps in plists)

    Tp = 1
    while any(rows_needed(pl, Tp) > 128 for pl in core_pairs):
        Tp += 1
    nact = min(Tp - 1, max(1, (2 * Tp) // 3))

    in_maps = []
    for c in range(NCORES):
        base = c * A
        repa = repb[base:base + A]
        repat = np.ascontiguousarray(
            repa.T.reshape(2, 128, A).transpose(1, 0, 2)
        )
        bigm = np.where(same[base:base + A], BIGM, 0.0).astype(ml_dtypes.bfloat16)
        dups = np.zeros((A, 128), ml_dtypes.bfloat16)
        idx = np.zeros((128, Tp), np.uint16)
        msk = np.zeros((128, Tp, 16), ml_dtypes.bfloat16)
        for p in range(128):
            msk[p, :, p % 16] = 1.0
        pmadj = np.full((128, Tp), -BIG, np.float32)
        r = 0
        for j, ps in enumerate(core_pairs[c]):
            for s in range(0, len(ps), Tp):
                chunk = ps[s:s + Tp]
                dups[j, r] = 1.0
                for t, p in enumerate(chunk):
                    idx[r, t] = p
                    pmadj[r, t] = MARGIN - BIGM
                r += 1
        assert r <= 128, (c, r)
        in_maps.append({
            "rept": rept,
            "repat": repat,
            "repa": repa,
            "bigm": bigm,
            "dups": dups,
            "idx": idx,
            "msk": msk,
            "pmadj": pmadj,
        })
    return Tp, nact, in_maps


def _run(rep, labels, trace=False):
    Tp, nact, in_maps = _prep(rep, labels)
    ndve = Tp - nact
    if (Tp, nact) not in _cache:
        _cache[(Tp, nact)] = _build(Tp, nact)
    nc = _cache[(Tp, nact)]
    res = run_bass_kernel_spmd(nc, in_maps, list(range(NCORES)), trace=trace)
    outs = np.stack([res.results[c]["out"] for c in range(NCORES)])  # [8, 2, NW]
    S = float(outs[:, 0, :ndve].sum()) + float(outs[:, 1, ndve:Tp].sum())
    C = float(outs[:, 1, Tp:].sum())
    loss = np.float32(S / (C + EPS))
    return np.asarray(loss, dtype=np.float32), res


def kernel(rep, labels):
    loss, _ = _run(rep, labels, trace=False)
    return loss
